# revision 17
# baseline (speedup 1.0000x reference)
"""CenterLoss forward on 8 Trainium2 NeuronCores.

Reference semantics:
    distmat[b, c] = ||x_b||^2 + ||center_c||^2 - 2 <x_b, center_c>
    loss = sum(clip(distmat * onehot(labels), 1e-12, 1e12)) / B

The masked matrix is zero everywhere except (b, labels[b]), and clip() lifts
each of the B*(C-1) zeros to exactly 1e-12.  So:

    loss = ( sum_b clip(||x_b - centers[labels[b]]||^2, 1e-12, 1e12)
             + B*(C-1)*1e-12 ) / B

which needs only a row gather + per-row squared distance, not the full
(B, C) distance matrix (42 GFLOP -> ~4 MFLOP).

Device kernel (raw Bass, single basic block, SPMD data-parallel over batch),
v6 — engineered from HW traces + the SWDGE cost model (994 ns fixed +
0.34 ns/descriptor per DMA instruction, receipts ~1-2 us after the data
lands, HBM effective bandwidth strongly descriptor-size-dependent):

  - shard row r maps to (partition, chunk) = (r // 4, r % 4), so a core's
    x shard is ONE HWDGE DMA whose descriptors are 4 consecutive rows =
    8 KB contiguous per partition (near line rate), instead of 4 chunked
    DMAs of 2 KB descriptors.  labels [128, 4] is the natural reshape.
  - centers are baked into the NEFF as a Const tensor in bf16, augmented
    with col 512 = ||c||^2 (rows padded to 520 cols, 1040 B, 16B-aligned
    descriptors).  One gathered row delivers the center AND its squared
    norm; bf16 halves the gather's HBM traffic vs f32.
  - label load goes out FIRST on the sync/HWDGE queue, then the x load.
  - per core, 4 indirect-DMA gathers of 128 rows, one per SWDGE queue
    (HW-verified: indirect_dma_start consumes only ONE index per
    partition; dma_gather would do 512 in one instruction but needs a Q7
    library reload this walrus build cannot compile).  Tiny 1-descriptor
    warm-up DMAs on each queue absorb the first-SWDGE-use warm-up (~1 us)
    during the label-DMA wait; tiny trailing flushers push the gathers'
    completion receipts while Q7 idles.
  - DVE per chunk: square-accumulate ||x||^2 while the gathers are in
    flight, then one fused 512-col scalar_tensor_tensor
        out = (c * -2) * x,   acc = sum(out) = -2<x, c>
    plus two [128, 1] micro-ops folding + ||c||^2 + ||x||^2 (off the
    critical tail for all but the last chunk).
  - result [128, 4] goes back via the idle sync/HWDGE queue; the clip and
    the analytic floor B*(C-1)*1e-12 are applied host-side along with the
    final sum (host already owned the cross-core reduction).
  - sync rules (sim race detector + hardware):
      * every DMA whose completion matters gets its own semaphore;
      * SWDGE sems are never shared with HWDGE DMAs;
      * same-engine RAW on DVE gets an explicit sem edge (dve_sem chain).
"""

import hashlib
from contextlib import ExitStack

import ml_dtypes
import numpy as np

import concourse.bass as bass
from concourse import mybir
from concourse.bass_utils import run_bass_kernel_spmd

B = 4096
D = 512
C = 10000
NCORES = 8
BL = B // NCORES          # 512 rows per core
P = 128                   # partitions
NT = BL // P              # 4 chunks per core

CW = 520                  # bf16 cols per baked centers row (16B-aligned stride)

F32 = mybir.dt.float32
BF16 = mybir.dt.bfloat16
I32 = mybir.dt.int32

_CACHE = {}


def legalize_waits(nc, max_waits=1):
    """The walrus build in this container accepts at most one embedded
    sem-wait per TPB instruction ("Too many sync wait commands" otherwise).
    Split any excess into standalone single-wait InstEventSemaphore no-ops
    immediately before the instruction on the same engine — engine program
    order then enforces the identical synchronization."""
    n_split = 0
    for f in nc.m.functions:
        for b in f.blocks:
            insts = list(b.instructions)
            out = []
            for inst in insts:
                si = inst.sync_info
                waits = list(si.on_wait) if (si is not None and si.on_wait) else []
                if len(waits) > max_waits:
                    keep = waits[-max_waits:]
                    spill = waits[:-max_waits]
                    for k, w in enumerate(spill):
                        out.append(
                            mybir.InstEventSemaphore(
                                name=f"{inst.name}-lw{k}",
                                engine=inst.engine,
                                sync_info=mybir.SyncInfo(on_wait=[w], on_update=[]),
                            )
                        )
                        n_split += 1
                    inst.sync_info = mybir.SyncInfo(
                        on_wait=keep, on_update=list(si.on_update or [])
                    )
                out.append(inst)
            b.instructions = out
    return n_split


def make_caug(centers_np):
    """bf16 [C, CW]: cols 0..D-1 = centers, col D = ||c||^2, rest 0."""
    c64 = np.asarray(centers_np, dtype=np.float64)
    csq = (c64 * c64).sum(axis=1)
    caug = np.zeros((C, CW), dtype=np.float32)
    caug[:, :D] = centers_np
    caug[:, D] = csq.astype(np.float32)
    return np.ascontiguousarray(caug.astype(ml_dtypes.bfloat16))


def build_nc(centers_np):
    nc = bass.Bass(num_swdge_queues=4)
    queues = ["qPoolDynamic", "qPoolDynamic1", "qPoolDynamic2", "qPoolDynamic3"]

    # shard row 4p + t lives at [p, t, :]
    x = nc.dram_tensor("x", [P, NT, D], F32, kind="ExternalInput")
    labels = nc.dram_tensor("labels", [P, NT], I32, kind="ExternalInput")
    out = nc.dram_tensor("out", [P, NT], F32, kind="ExternalOutput")
    caug = nc.inline_tensor(make_caug(centers_np), name="caug")

    es = ExitStack()
    idx_sb = es.enter_context(nc.sbuf_tensor("idx_sb", [P, NT], I32))
    x_sb = es.enter_context(nc.sbuf_tensor("x_sb", [P, NT * D], F32))
    c_sb = es.enter_context(nc.sbuf_tensor("c_sb", [P, NT, CW], BF16))
    sq_sb = es.enter_context(nc.sbuf_tensor("sq_sb", [P, NT * D], F32))
    prod_sb = es.enter_context(nc.sbuf_tensor("prod_sb", [P, NT * D], F32))
    xsq_sb = es.enter_context(nc.sbuf_tensor("xsq_sb", [P, NT], F32))
    acc_sb = es.enter_context(nc.sbuf_tensor("acc_sb", [P, NT], F32))
    s_sb = es.enter_context(nc.sbuf_tensor("s_sb", [P, NT], F32))
    dist_sb = es.enter_context(nc.sbuf_tensor("dist_sb", [P, NT], F32))
    scr_sb = es.enter_context(nc.sbuf_tensor("scr_sb", [P, NT], I32))
    idx_sem = es.enter_context(nc.semaphore("idx_sem"))
    x_sem = es.enter_context(nc.semaphore("x_sem"))
    c_sems = [es.enter_context(nc.semaphore(f"c_sem{t}")) for t in range(NT)]
    v_sem = es.enter_context(nc.semaphore("v_sem"))
    o_sem = es.enter_context(nc.semaphore("o_sem"))
    dve_sem = es.enter_context(nc.semaphore("dve_sem"))
    f_sem = es.enter_context(nc.semaphore("f_sem"))

    # ---- sync/HWDGE: labels first (they gate the gathers), then the x
    # shard as ONE DMA (8 KB contiguous per partition) ----
    nc.sync.dma_start(out=idx_sb[:, :], in_=labels[:, :]).then_inc(idx_sem, 16)
    nc.sync.dma_start(out=x_sb[:, :], in_=x[:, :, :]).then_inc(x_sem, 16)

    # ---- gpsimd: tiny warm-up DMAs on all SWDGE queues while the label
    # DMA is in flight (first SWDGE use after idle pays ~1 us), then the
    # four gathers, then tiny receipt flushers ----
    for q in range(4):
        wi = nc.gpsimd.dma_start(
            out=scr_sb[0:1, :], in_=labels[0:1, :]
        ).then_inc(f_sem, 16)
        wi.ins.queue = queues[q]
    nc.gpsimd.wait_ge(idx_sem, 16)
    for t in range(NT):
        gi = nc.gpsimd.indirect_dma_start(
            out=c_sb[:, t, :],
            out_offset=None,
            in_=caug[:],
            in_offset=bass.IndirectOffsetOnAxis(ap=idx_sb[:, t:t + 1], axis=0),
        ).then_inc(c_sems[t], 16)
        gi.ins.queue = queues[t]
    for q in range(4):
        fi = nc.gpsimd.dma_start(
            out=scr_sb[0:1, :], in_=labels[0:1, :]
        ).then_inc(f_sem, 16)
        fi.ins.queue = queues[q]

    # ---- vector (DVE) ----
    n_dve = 0
    # ||x||^2 per chunk while the gathers are still in flight
    nc.vector.wait_ge(x_sem, 16)
    for t in range(NT):
        xc = x_sb[:, t * D:(t + 1) * D]
        nc.vector.scalar_tensor_tensor(
            out=sq_sb[:, t * D:(t + 1) * D],
            in0=xc,
            scalar=1.0,
            in1=xc,
            op0=mybir.AluOpType.mult,
            op1=mybir.AluOpType.mult,
            accum_out=xsq_sb[:, t:t + 1],
        ).then_inc(dve_sem, 1)
        n_dve += 1
    # per chunk: acc = sum((c * -2) * x) = -2<x, c>, then fold in
    # ||c||^2 (gathered, col D) and ||x||^2 via two [128, 1] micro-ops
    for t in range(NT):
        nc.vector.wait_ge(c_sems[t], 16)
        nc.vector.wait_ge(dve_sem, n_dve)  # RAW edges (xsq, prior acc)
        nc.vector.scalar_tensor_tensor(
            out=prod_sb[:, t * D:(t + 1) * D],
            in0=c_sb[:, t, 0:D],
            scalar=-2.0,
            in1=x_sb[:, t * D:(t + 1) * D],
            op0=mybir.AluOpType.mult,
            op1=mybir.AluOpType.mult,
            accum_out=acc_sb[:, t:t + 1],
        ).then_inc(dve_sem, 1)
        n_dve += 1
        nc.vector.wait_ge(dve_sem, n_dve)
        nc.vector.scalar_tensor_tensor(
            out=s_sb[:, t:t + 1],
            in0=c_sb[:, t, D:D + 1],
            scalar=1.0,
            in1=xsq_sb[:, t:t + 1],
            op0=mybir.AluOpType.mult,
            op1=mybir.AluOpType.add,
        ).then_inc(dve_sem, 1)
        n_dve += 1
        nc.vector.wait_ge(dve_sem, n_dve)
        ti = nc.vector.scalar_tensor_tensor(
            out=dist_sb[:, t:t + 1],
            in0=acc_sb[:, t:t + 1],
            scalar=1.0,
            in1=s_sb[:, t:t + 1],
            op0=mybir.AluOpType.mult,
            op1=mybir.AluOpType.add,
        )
        if t == NT - 1:
            ti.then_inc(v_sem, 1)
        else:
            ti.then_inc(dve_sem, 1)
            n_dve += 1

    # ---- result out via the idle sync/HWDGE queue ----
    nc.sync.wait_ge(v_sem, 1)
    nc.sync.dma_start(out=out[:, :], in_=dist_sb[:, :]).then_inc(o_sem, 16)

    # NOTE: the ExitStack is intentionally NOT closed — closing would free
    # the semaphores and emit an expensive end-of-program drain + barrier;
    # the NEFF-level postamble already clears the kernel sem range, so
    # repeated executions stay safe without it.
    legalize_waits(nc)
    return nc


def _get_nc(centers_np):
    arr = np.ascontiguousarray(centers_np, np.float32)
    key = hashlib.md5(arr.tobytes()).hexdigest()
    if _CACHE.get("key") != key:
        _CACHE["nc"] = build_nc(arr)
        _CACHE["key"] = key
    return _CACHE["nc"]


def make_in_maps(x, labels, centers=None):
    # shard row r = 4p + t -> x[p, t, :], labels[p, t]: natural reshapes
    x = np.ascontiguousarray(
        np.asarray(x, dtype=np.float32).reshape(NCORES, P, NT, D)
    )
    labels_i32 = np.ascontiguousarray(
        np.asarray(labels).astype(np.int32).reshape(NCORES, P, NT)
    )
    return [{"x": x[i], "labels": labels_i32[i]} for i in range(NCORES)]


def finalize(results):
    total = 0.0
    for r in results:
        d = np.asarray(r["out"], dtype=np.float64)
        total += float(np.clip(d, 1e-12, 1e12).sum())
    loss = (total + B * (C - 1) * 1e-12) / B
    return np.array(loss, dtype=np.float32)


def kernel(x, labels, centers):
    nc = _get_nc(centers)
    in_maps = make_in_maps(x, labels)
    res = run_bass_kernel_spmd(nc, in_maps, core_ids=list(range(NCORES)))
    return finalize(res.results)


# revision 18
# speedup vs baseline: 1.2294x; 1.2294x over previous
"""CenterLoss forward on 8 Trainium2 NeuronCores.

Reference semantics:
    distmat[b, c] = ||x_b||^2 + ||center_c||^2 - 2 <x_b, center_c>
    loss = sum(clip(distmat * onehot(labels), 1e-12, 1e12)) / B

The masked matrix is zero everywhere except (b, labels[b]), and clip() lifts
each of the B*(C-1) zeros to exactly 1e-12.  So:

    loss = ( sum_b clip(||x_b - centers[labels[b]]||^2, 1e-12, 1e12)
             + B*(C-1)*1e-12 ) / B

which needs only a row gather + per-row squared distance, not the full
(B, C) distance matrix (42 GFLOP -> ~4 MFLOP).

Device kernel (raw Bass, single basic block, SPMD data-parallel over batch),
v7 — engineered from HW traces + the SWDGE cost model (994 ns fixed +
0.34 ns/descriptor per DMA instruction, completion receipts ~1-2 us after
the data lands, HBM effective bandwidth strongly descriptor-size-dependent):

  - shard row r maps to (partition, chunk) = (r // 4, r % 4), so the x
    shard loads as TWO HWDGE DMAs whose descriptors are 2 consecutive
    rows = 4 KB contiguous per partition, and the first half lands early
    enough to hide the ||x||^2 squares inside the gather window.
  - centers are baked into the NEFF as a Const tensor in bf16, augmented
    with col 512 = ||c||^2 (rows padded to 520 cols, 1040 B descriptors).
    One gathered row delivers the center AND its squared norm; bf16
    halves the gather's HBM traffic vs f32.
  - label load goes out FIRST on the sync/HWDGE queue, then the x halves.
  - per core, 4 indirect-DMA gathers of 128 rows alternating across the
    two SWDGE queues (HW-verified: indirect_dma_start consumes only ONE
    index per partition and the wait->first-gather dispatch gap (~0.9 us)
    is inherent, not a warm-up effect; dma_gather would do 512 rows in
    one instruction but needs a Q7 library reload this walrus build
    cannot compile).  Tiny trailing flusher DMAs push the gathers'
    completion receipts through while Q7 idles.
  - DVE per chunk: ||x||^2 square-accumulates run inside the gather
    window; then per chunk S_t = ||c||^2 + ||x||^2 (micro-op, before the
    dot so it's off the tail), one fused 512-col scalar_tensor_tensor
        out = (c * -2) * x,   acc = sum(out) = -2<x, c>
    and dist_t = acc_t + S_t (micro-op).
  - the Bass-init all-engine barrier and its const-pool memsets are
    stripped post-hoc: nothing in this kernel reads the const APs, every
    cross-engine dependency is already carried by explicit semaphores,
    and the NEFF postamble re-clears the sem range each run — each engine
    can enter its kernel code straight from its register preamble
    (~0.6 us earlier).
  - result [128, 4] goes back via the idle sync/HWDGE queue; the clip and
    the analytic floor B*(C-1)*1e-12 are applied host-side along with the
    final sum (host already owned the cross-core reduction).
  - sync rules (sim race detector + hardware):
      * every DMA whose completion matters gets its own semaphore;
      * SWDGE sems are never shared with HWDGE DMAs;
      * same-engine RAW on DVE gets an explicit sem edge (dve_sem chain).
"""

import hashlib
from contextlib import ExitStack

import ml_dtypes
import numpy as np

import concourse.bass as bass
from concourse import mybir
from concourse.bass_utils import run_bass_kernel_spmd

B = 4096
D = 512
C = 10000
NCORES = 8
BL = B // NCORES          # 512 rows per core
P = 128                   # partitions
NT = BL // P              # 4 chunks per core

CW = 520                  # bf16 cols per baked centers row (16B-aligned stride)

F32 = mybir.dt.float32
BF16 = mybir.dt.bfloat16
I32 = mybir.dt.int32

_CACHE = {}


def legalize_waits(nc, max_waits=1):
    """The walrus build in this container accepts at most one embedded
    sem-wait per TPB instruction ("Too many sync wait commands" otherwise).
    Split any excess into standalone single-wait InstEventSemaphore no-ops
    immediately before the instruction on the same engine — engine program
    order then enforces the identical synchronization."""
    n_split = 0
    for f in nc.m.functions:
        for b in f.blocks:
            insts = list(b.instructions)
            out = []
            for inst in insts:
                si = inst.sync_info
                waits = list(si.on_wait) if (si is not None and si.on_wait) else []
                if len(waits) > max_waits:
                    keep = waits[-max_waits:]
                    spill = waits[:-max_waits]
                    for k, w in enumerate(spill):
                        out.append(
                            mybir.InstEventSemaphore(
                                name=f"{inst.name}-lw{k}",
                                engine=inst.engine,
                                sync_info=mybir.SyncInfo(on_wait=[w], on_update=[]),
                            )
                        )
                        n_split += 1
                    inst.sync_info = mybir.SyncInfo(
                        on_wait=keep, on_update=list(si.on_update or [])
                    )
                out.append(inst)
            b.instructions = out
    return n_split


def strip_init_barrier(nc):
    """Remove the Bass-init all-engine barrier (Drain + EventSemaphore per
    engine) and the const-pool memsets.  Safe here: the kernel never reads
    the const APs, all cross-engine edges are explicit sems that start at
    0 (the NEFF postamble clears the kernel sem range every run), and no
    engine's kernel code depends on another engine's register preamble."""
    n = 0
    for f in nc.m.functions:
        for b in f.blocks:
            keep = []
            for i in b.instructions:
                drop = False
                if (i.name or "").startswith("barrier_"):
                    drop = True
                si = i.sync_info
                if si is not None:
                    for w in list(si.on_wait or []) + list(si.on_update or []):
                        if "barrier_" in (w.ant_name or ""):
                            drop = True
                if isinstance(i, mybir.InstMemset):
                    o = i.outs[0]
                    if str(getattr(o, "memref", "")).startswith("const-"):
                        drop = True
                if drop:
                    n += 1
                else:
                    keep.append(i)
            b.instructions = keep
    return n


def make_caug(centers_np):
    """bf16 [C, CW]: cols 0..D-1 = centers, col D = ||c||^2, rest 0."""
    c64 = np.asarray(centers_np, dtype=np.float64)
    csq = (c64 * c64).sum(axis=1)
    caug = np.zeros((C, CW), dtype=np.float32)
    caug[:, :D] = centers_np
    caug[:, D] = csq.astype(np.float32)
    return np.ascontiguousarray(caug.astype(ml_dtypes.bfloat16))


def build_nc(centers_np):
    nc = bass.Bass(num_swdge_queues=2)

    # shard row 4p + t lives at [p, t, :]
    x = nc.dram_tensor("x", [P, NT, D], F32, kind="ExternalInput")
    labels = nc.dram_tensor("labels", [P, NT], I32, kind="ExternalInput")
    out = nc.dram_tensor("out", [P, NT], F32, kind="ExternalOutput")
    caug = nc.inline_tensor(make_caug(centers_np), name="caug")

    es = ExitStack()
    idx_sb = es.enter_context(nc.sbuf_tensor("idx_sb", [P, NT], I32))
    x_sb = es.enter_context(nc.sbuf_tensor("x_sb", [P, NT * D], F32))
    c_sb = es.enter_context(nc.sbuf_tensor("c_sb", [P, NT, CW], BF16))
    sq_sb = es.enter_context(nc.sbuf_tensor("sq_sb", [P, NT * D], F32))
    prod_sb = es.enter_context(nc.sbuf_tensor("prod_sb", [P, NT * D], F32))
    xsq_sb = es.enter_context(nc.sbuf_tensor("xsq_sb", [P, NT], F32))
    acc_sb = es.enter_context(nc.sbuf_tensor("acc_sb", [P, NT], F32))
    s_sb = es.enter_context(nc.sbuf_tensor("s_sb", [P, NT], F32))
    dist_sb = es.enter_context(nc.sbuf_tensor("dist_sb", [P, NT], F32))
    scr_sb = es.enter_context(nc.sbuf_tensor("scr_sb", [P, NT], I32))
    idx_sem = es.enter_context(nc.semaphore("idx_sem"))
    xh_sems = [es.enter_context(nc.semaphore(f"xh_sem{h}")) for h in range(2)]
    c_sems = [es.enter_context(nc.semaphore(f"c_sem{t}")) for t in range(NT)]
    v_sem = es.enter_context(nc.semaphore("v_sem"))
    o_sem = es.enter_context(nc.semaphore("o_sem"))
    dve_sem = es.enter_context(nc.semaphore("dve_sem"))
    f_sem = es.enter_context(nc.semaphore("f_sem"))

    # ---- sync/HWDGE: labels first (they gate the gathers), then the x
    # shard in two halves (4 KB contiguous per partition each) ----
    nc.sync.dma_start(out=idx_sb[:, :], in_=labels[:, :]).then_inc(idx_sem, 16)
    for h in range(2):
        nc.sync.dma_start(
            out=x_sb[:, h * 2 * D:(h + 1) * 2 * D],
            in_=x[:, 2 * h:2 * h + 2, :],
        ).then_inc(xh_sems[h], 16)

    # ---- gpsimd: four 128-row gathers alternating across the SWDGE
    # queues, then tiny receipt flushers ----
    nc.gpsimd.wait_ge(idx_sem, 16)
    for t in range(NT):
        gi = nc.gpsimd.indirect_dma_start(
            out=c_sb[:, t, :],
            out_offset=None,
            in_=caug[:],
            in_offset=bass.IndirectOffsetOnAxis(ap=idx_sb[:, t:t + 1], axis=0),
        ).then_inc(c_sems[t], 16)
        if t % 2 == 1:
            gi.ins.queue = "qPoolDynamic1"
    for q in range(2):
        fi = nc.gpsimd.dma_start(
            out=scr_sb[0:1, :], in_=labels[0:1, :]
        ).then_inc(f_sem, 16)
        if q == 1:
            fi.ins.queue = "qPoolDynamic1"

    # ---- vector (DVE) ----
    n_dve = 0
    # ||x||^2 per chunk while the gathers are still in flight
    for t in range(NT):
        xc = x_sb[:, t * D:(t + 1) * D]
        nc.vector.wait_ge(xh_sems[t // 2], 16)
        nc.vector.scalar_tensor_tensor(
            out=sq_sb[:, t * D:(t + 1) * D],
            in0=xc,
            scalar=1.0,
            in1=xc,
            op0=mybir.AluOpType.mult,
            op1=mybir.AluOpType.mult,
            accum_out=xsq_sb[:, t:t + 1],
        ).then_inc(dve_sem, 1)
        n_dve += 1
    # per chunk: S = ||c||^2 + ||x||^2 (before the dot - off the tail),
    # acc = sum((c * -2) * x) = -2<x, c>, dist = acc + S
    for t in range(NT):
        nc.vector.wait_ge(c_sems[t], 16)
        nc.vector.wait_ge(dve_sem, n_dve)  # RAW edges (xsq, prior chunk)
        nc.vector.scalar_tensor_tensor(
            out=s_sb[:, t:t + 1],
            in0=c_sb[:, t, D:D + 1],
            scalar=1.0,
            in1=xsq_sb[:, t:t + 1],
            op0=mybir.AluOpType.mult,
            op1=mybir.AluOpType.add,
        ).then_inc(dve_sem, 1)
        n_dve += 1
        nc.vector.scalar_tensor_tensor(
            out=prod_sb[:, t * D:(t + 1) * D],
            in0=c_sb[:, t, 0:D],
            scalar=-2.0,
            in1=x_sb[:, t * D:(t + 1) * D],
            op0=mybir.AluOpType.mult,
            op1=mybir.AluOpType.mult,
            accum_out=acc_sb[:, t:t + 1],
        ).then_inc(dve_sem, 1)
        n_dve += 1
        nc.vector.wait_ge(dve_sem, n_dve)
        ti = nc.vector.scalar_tensor_tensor(
            out=dist_sb[:, t:t + 1],
            in0=acc_sb[:, t:t + 1],
            scalar=1.0,
            in1=s_sb[:, t:t + 1],
            op0=mybir.AluOpType.mult,
            op1=mybir.AluOpType.add,
        )
        if t == NT - 1:
            ti.then_inc(v_sem, 1)
        else:
            ti.then_inc(dve_sem, 1)
            n_dve += 1

    # ---- result out via the idle sync/HWDGE queue ----
    nc.sync.wait_ge(v_sem, 1)
    nc.sync.dma_start(out=out[:, :], in_=dist_sb[:, :]).then_inc(o_sem, 16)

    # NOTE: the ExitStack is intentionally NOT closed — closing would free
    # the semaphores and emit an expensive end-of-program drain + barrier;
    # the NEFF-level postamble already clears the kernel sem range, so
    # repeated executions stay safe without it.
    strip_init_barrier(nc)
    legalize_waits(nc)
    return nc


def _get_nc(centers_np):
    arr = np.ascontiguousarray(centers_np, np.float32)
    key = hashlib.md5(arr.tobytes()).hexdigest()
    if _CACHE.get("key") != key:
        _CACHE["nc"] = build_nc(arr)
        _CACHE["key"] = key
    return _CACHE["nc"]


def make_in_maps(x, labels, centers=None):
    # shard row r = 4p + t -> x[p, t, :], labels[p, t]: natural reshapes
    x = np.ascontiguousarray(
        np.asarray(x, dtype=np.float32).reshape(NCORES, P, NT, D)
    )
    labels_i32 = np.ascontiguousarray(
        np.asarray(labels).astype(np.int32).reshape(NCORES, P, NT)
    )
    return [{"x": x[i], "labels": labels_i32[i]} for i in range(NCORES)]


def finalize(results):
    total = 0.0
    for r in results:
        d = np.asarray(r["out"], dtype=np.float64)
        total += float(np.clip(d, 1e-12, 1e12).sum())
    loss = (total + B * (C - 1) * 1e-12) / B
    return np.array(loss, dtype=np.float32)


def kernel(x, labels, centers):
    nc = _get_nc(centers)
    in_maps = make_in_maps(x, labels)
    res = run_bass_kernel_spmd(nc, in_maps, core_ids=list(range(NCORES)))
    return finalize(res.results)
